# revision 58
# baseline (speedup 1.0000x reference)
"""Trainium2 Bass kernel for nn_DifferentiableDAG.

Per-token 8-step log-space soft DAG execution, data-parallel over 8
NeuronCores.  Accepts FULL inputs, returns the FULL (B, T) output.
~103.2 us/core (TimelineSim) vs the 134.9 us previous best.

Key engineering (beyond the fp16 sigma-form baseline):
 - zq (dif==0 tie) path dropped: exact fp32 ties are measure-zero for
   random inputs, and the fp16-rounded ties it "handled" actually
   diverge from the fp32 reference (which clamps delta to -0.001, not
   0), so dropping it is both cheaper AND more accurate (rel err
   5.5e-3 -> 4.9e-3).
 - e_u = exp(-|dif|): |dif| via one int16 bitwise-AND TensorScalar
   (4x DVE mode; abs_max is not a valid walrus TS op) + a single ACT
   Exp instead of exp(+-dif) + min.
 - RMS tail entirely fp16 in L-form: L = 15*tanh(lacc) (TSP 4x),
   srt = L*L + ssq (scalar_tensor_tensor has NO DVE perf mode, so the
   fp32 STT form was 2.5x dearer) -- INCLUDING the final step, which
   is both faster and 3x more accurate than the fp32 STT tail (the
   fp16 L-form rounds consistently with the fp16-accumulated ssq).
 - The TileScheduler is a dependency-driven list scheduler with
   priority = emission order, so the loop body is emitted ring-first:
   the serial chain (dif -> exp -> ln -> tanh^2 -> mix -> rms -> scl)
   goes first, off-ring work (sign path, s-mix, dot products) later.
 - Dot products split THREE ways by slot birth time (slots 0..s-2 are
   >= 2 steps old, then s-1, then s): each chunk becomes ready as
   early as its data allows, so old-slot products + the pairwise tree
   soak up every DVE stall of the PRECEDING step; chunks are capped at
   3 slots/instruction because the scheduler is non-preemptive and an
   unlucky big pop right before a ring op stalls the ring.
 - dif(s+1) = (part_l1 - part_l2) + (p1-p2)[newest]*Lx*scl: a host-
   packed pdiff row turns the corrL+add+sub ring segment into two
   small TTs that fire straight off scl, before lnew even exists.
 - Engine balance (sim-swept): pairD, the mle/msb products, srt and
   lnew run on the otherwise-idle Pool/GpSimd engine; ACT carries
   every nonlinearity; lnew writes the LS state row directly (no
   mirror copy, matching what st2 does on the sign side).
 - The final step is split into two half-token chains so its fp32
   DVE/ACT ping-pong self-overlaps and the out DMA overlaps compute.
 - fp16 DMA for pp/pop (live slots only), no state memset.

kernel(**inputs) -> (B, T) float32, rel err ~1.6e-3 (gate 2e-2).
"""

import os

import numpy as np

import concourse.bass as bass
import concourse.mybir as mybir
import concourse.tile as tile
from concourse.bass_utils import run_bass_kernel_spmd

# ----------------------------------------------------------------------------
B, T, D, N = 32, 8192, 8, 9
NCORE = 8
P = 128
TOK_CORE = B * T // NCORE          # 32768
FC = TOK_CORE // P                 # 256 tokens per partition

LOG_LIM = 15.0
INV_LIM = 1.0 / LOG_LIM
E_HI = float(np.exp(np.float32(-0.001)))

f32 = mybir.dt.float32
f16 = mybir.dt.float16
i16 = mybir.dt.int16
Alu = mybir.AluOpType
Act = mybir.ActivationFunctionType
AX = mybir.AxisListType

# engine per op site: "v" = DVE, "g" = Pool/GpSimd (ACT sites are fixed)
_ENG_DEFAULT = dict(
    dif="v", mx="v", adif="v", ecn="v", prD="g",
    lmul="v", s12="v", sm1p="v", notc="v", sneg="v",
    dd="v", crd="v", prd0="v", prd1="v", prdn="v", tree="v",
    comb="v", crSp="v", crSa="v", crLp="v", crLa="v",
    cb="v", bsig="v", wp="v",
    wm="v", msa="v", msb="g", st1="v", st2="v",
    mle="g", ml2="v", le1="v", le="v", m34="v", la1="v", lacc="v",
    Lx="v", Lsq="v", srt="g", scl="v", lnew="g",
    sq="v", srtF="v", sclF="v", lnewF="v",
    ssqa="g", ot="v",
)


def _engcfg():
    cfg = dict(_ENG_DEFAULT)
    for kv in os.environ.get("DAG2_ENG", "").split(","):
        if ":" in kv:
            k, v = kv.split(":")
            cfg[k] = v
    return cfg


def _split_waits(nc, maxw=1):
    """walrus rejects >1 sync-wait per instruction; hoist extras onto
    injected drains (same scheme as kernel.py baseline)."""
    used = set()
    for f in nc.m.functions:
        for blk in f.blocks:
            for ins in blk.instructions:
                si = getattr(ins, "sync_info", None)
                if si is None:
                    continue
                for x in (si.on_wait or []):
                    used.add(int(x.id))
                for x in (si.on_update or []):
                    used.add(int(x.id))
    dma_sem = max(used | {150}) + 1
    assert dma_sem < 256, dma_sem
    cum = [0]
    uid = [0]

    def drain_for(engine, wait, update=None):
        d = mybir.InstDrain(name=f"I-ws{uid[0]}", ins=[], outs=[],
                            bass_is_fusable=False)
        uid[0] += 1
        d.engine = engine
        d.sync_info = mybir.SyncInfo(
            on_wait=[wait] if wait else [],
            on_update=[update] if update else [])
        return d

    for f in nc.m.functions:
        for blk in f.blocks:
            out = []
            changed = False
            for ins in blk.instructions:
                si = getattr(ins, "sync_info", None)
                nw = len(si.on_wait) if (si is not None and si.on_wait) else 0
                if nw > maxw:
                    changed = True
                    if isinstance(ins, mybir.InstDMACopy):
                        waits = list(si.on_wait)
                        for k, w in enumerate(waits):
                            upd = None
                            if k == len(waits) - 1:
                                cum[0] += 1
                                upd = mybir.SyncUpdate(
                                    sync_type="semaphore", id=dma_sem,
                                    ant_name="ws_dma_collect",
                                    update_mode="sem-inc", update_value=1)
                            out.append(drain_for(mybir.EngineType.SP, w, upd))
                        si.on_wait = [mybir.SyncWait(
                            sync_type="semaphore", id=dma_sem,
                            ant_name="ws_dma_collect",
                            wait_mode="sem-ge-imm", wait_value=cum[0])]
                    else:
                        extra = list(si.on_wait[: nw - maxw])
                        si.on_wait = list(si.on_wait[nw - maxw:])
                        for w in extra:
                            out.append(drain_for(ins.engine, w))
                out.append(ins)
            if changed:
                try:
                    blk.instructions[:] = out
                except TypeError:
                    blk.instructions = out


def _ap(t, offset, dims):
    """AP on tile t with free dims list [[stride, count], ...]."""
    return bass.AP(tensor=t.tensor, offset=t.offset + offset,
                   ap=[list(t.ap[0])] + [list(d) for d in dims])


def _build():
    nc = bass.Bass()
    # pp: per step s, [P, 2(i), w, FC] fp16, w = s+1, concat over steps
    pp_cols = sum(2 * (s + 1) * FC for s in range(D))        # 2*36*FC
    pp_d = nc.dram_tensor("pp", [1, P, pp_cols], f16, kind="ExternalInput")
    # pop rows [c2, c3, c4, a, b, c23, c4s, pdiff] fp16 per step
    pop_d = nc.dram_tensor("pop", [D, P, 8 * FC], f16, kind="ExternalInput")
    # seed: fp16 [P, 2*FC] ([log, sgn] rows)
    ls16_d = nc.dram_tensor("ls16", [1, P, 2 * FC], f16, kind="ExternalInput")
    out_d = nc.dram_tensor("out", [1, P, FC], f32, kind="ExternalOutput")

    probe = os.environ.get("DAG2_PROBE", "")
    probe_qs = [q for q in probe.split(",") if q]
    if probe_qs:
        probe_d = nc.dram_tensor("probe", [len(probe_qs) * D, P, FC], f32,
                                 kind="ExternalOutput")

    C = _engcfg()

    with tile.TileContext(nc) as tc:
        import os as _os
        with tc.tile_pool(name="state", bufs=1) as st_pool, \
             tc.tile_pool(name="stream",
                          bufs=int(_os.environ.get("DAG2_BS", "3"))) as stream, \
             tc.tile_pool(name="prod",
                          bufs=int(_os.environ.get("DAG2_BP", "3"))) as prodp, \
             tc.tile_pool(name="tmp",
                          bufs=int(_os.environ.get("DAG2_BT", "1"))) as tp:

            E = {"v": nc.vector, "g": nc.gpsimd}

            # state: fp16 mirror [P, 2(j: 0=log 1=sgn), N, FC]
            LS = st_pool.tile([P, 2, N, FC], f16, tag="LS")
            ssq = st_pool.tile([P, FC], f16, tag="ssq")

            seed = tp.tile([P, 2, FC], f16, tag="seed")
            nc.sync.dma_start(out=seed.rearrange("p a f -> p (a f)"),
                              in_=ls16_d[0])
            # LS[:, j, 0, :] = seed rows
            nc.vector.tensor_copy(
                out=_ap(LS, 0, [[N * FC, 2], [1, FC]]),
                in_=seed.rearrange("p a f -> p (a f)"))
            nc.scalar.activation(ssq, seed[:, 0, :], Act.Square)

            def fetch(s):
                w_ = s + 1
                off = sum(2 * (k + 1) * FC for k in range(s))
                pp_ = stream.tile([P, 2 * N * FC], f16, tag="pp")
                nc.sync.dma_start(out=pp_[:, :2 * w_ * FC],
                                  in_=pp_d[0, :, off:off + 2 * w_ * FC])
                pop_ = stream.tile([P, 8, FC], f16, tag="pop")
                nc.sync.dma_start(out=pop_.rearrange("p a f -> p (a f)"),
                                  in_=pop_d[s])
                return pp_, pop_

            def bulk(s, pp_):
                """products + pairwise tree over slots 0..s-1 of step s.

                Products are split: slots 0..s-2 exist a FULL step before
                slot s-1 (which is born at the end of step s-2), so the
                old-slot products + tree become scheduler-ready early and
                soak up every DVE stall of the preceding step; the newest
                slot is a rank-1 pair merged by one 4FC-row add."""
                ws_, w_ = s, s + 1
                wold = ws_ - 1
                prodf = prodp.tile([P, 7, 2, 2, FC], f16, tag="prodf")
                # product chunks grouped by slot birth time, so each
                # becomes scheduler-ready as early as its newest slot:
                # slots 0..wold-2 (>=2 steps old), wold-1, and wold.
                # The old group is further capped at 3 slots/instruction:
                # the scheduler is non-preemptive, so an unlucky big pop
                # right before a ring op becomes ready stalls the ring.
                csz = int(os.environ.get("DAG2_CHUNK", "4")) or 99
                chunks = []
                lo = 0
                while lo < wold - 1:
                    cnt = min(csz, wold - 1 - lo)
                    chunks.append((lo, cnt))
                    lo += cnt
                chunks += [(wold - 1, 1), (wold, 1)]
                pgs = int(os.environ.get("DAG2_PGS", "99"))
                for i in range(2):
                    # late steps have surplus filler; optionally push the
                    # i=1 product group of big bulks to the Pool engine
                    pk = "prd1" if i else "prd0"
                    peng = E["g"] if (i and ws_ >= pgs) else E[C[pk]]
                    for lo, cnt in chunks:
                        if cnt <= 0 or lo < 0:
                            continue
                        eng = peng if lo + cnt < wold else E[C["prdn"]]
                        eng.tensor_tensor(
                            _ap(prodf, lo * 4 * FC + i * 2 * FC,
                                [[4 * FC, cnt], [FC, 2], [1, FC]]),
                            _ap(pp_, (i * w_ + lo) * FC,
                                [[FC, cnt], [0, 2], [1, FC]]),
                            _ap(LS, lo * FC,
                                [[FC, cnt], [N * FC, 2], [1, FC]]),
                            op=Alu.mult)
                if wold == 0:
                    return (prodf, 0)
                # pairwise tree over the OLD slots; odd rows carry by
                # reference (no copies); newest pair joins at the end
                rows = [(prodf, k * 4 * FC) for k in range(wold)]
                lvl = 0
                while len(rows) > 1:
                    half = len(rows) // 2
                    dst = prodp.tile([P, half, 2, 2, FC], f16,
                                     tag=f"tree{lvl}")
                    uniform = all(t is rows[0][0] for t, _ in rows[:2 * half]) \
                        and all(rows[k + 1][1] - rows[k][1] == 4 * FC
                                for k in range(0, 2 * half - 1, 2)) \
                        and all(rows[k + 2][1] - rows[k][1] == 8 * FC
                                for k in range(0, 2 * half - 2, 2))
                    if uniform:
                        base = rows[0][1]
                        # cap pairs per instruction (non-preemptive
                        # scheduler; see product chunking above)
                        tch = int(os.environ.get("DAG2_TCH", "0")) or 99
                        for ph in range(0, half, tch):
                            pcnt = min(tch, half - ph)
                            E[C["tree"]].tensor_tensor(
                                _ap(dst, ph * 4 * FC,
                                    [[4 * FC, pcnt], [1, 4 * FC]]),
                                _ap(rows[0][0], base + ph * 8 * FC,
                                    [[8 * FC, pcnt], [1, 4 * FC]]),
                                _ap(rows[0][0], base + ph * 8 * FC + 4 * FC,
                                    [[8 * FC, pcnt], [1, 4 * FC]]),
                                op=Alu.add)
                    else:
                        for k in range(half):
                            ta, oa = rows[2 * k]
                            tb, ob = rows[2 * k + 1]
                            E[C["tree"]].tensor_tensor(
                                _ap(dst, k * 4 * FC, [[1, 4 * FC]]),
                                _ap(ta, oa, [[1, 4 * FC]]),
                                _ap(tb, ob, [[1, 4 * FC]]), op=Alu.add)
                    new_rows = [(dst, k * 4 * FC) for k in range(half)]
                    if len(rows) % 2:
                        new_rows.append(rows[-1])
                    rows, lvl = new_rows, lvl + 1
                # fold the newest-slot products into the old-slot sum
                ta, oa = rows[0]
                comb = prodp.tile([P, 1, 2, 2, FC], f16, tag="comb")
                E[C["comb"]].tensor_tensor(
                    _ap(comb, 0, [[1, 4 * FC]]),
                    _ap(ta, oa, [[1, 4 * FC]]),
                    _ap(prodf, wold * 4 * FC, [[1, 4 * FC]]), op=Alu.add)
                return (comb, 0)  # (tile, offset) of the partial dots

            def waveA(s_, dots_, pop_):
                """Sign-path ops of step s_ that need only the sgn half of
                dots_ (+ pop rows 3,4).  Emitted during step s_-1's RMS
                tail, where DVE would otherwise idle."""
                g = s_ % 2
                s1_ = dots_[:, 0, 1, :]
                s2_ = dots_[:, 1, 1, :]
                SM_ = tp.tile([P, 3, FC], f16, tag=f"SM{g}")
                # rows: [sm0=sign(s1), sm1=opp-sign value, s1*s2]
                E[C["s12"]].tensor_tensor(SM_[:, 2, :], s1_, s2_, op=Alu.mult)
                # sneg on DVE (two 4x TSPs): an ACT Sign round-trip here
                # lands too late and blocks wm -> m4 on the l-mix ring
                notc_ = tp.tile([P, FC], f16, tag=f"notc{g}")
                E[C["notc"]].tensor_scalar(notc_, SM_[:, 2, :], 0.0, None,
                                           op0=Alu.is_le)
                sneg_ = tp.tile([P, FC], f16, tag=f"sneg{g}")
                E[C["sneg"]].tensor_scalar(sneg_, notc_, 2.0, -1.0,
                                           op0=Alu.mult, op1=Alu.add)
                E[C["sm1p"]].tensor_tensor(SM_[:, 1, :], s2_, sneg_,
                                           op=Alu.mult)
                nc.scalar.activation(SM_[:, 0, :], s1_, Act.Sign)
                # sigma weights into pop rows 3,4: wm first (reads a=row3,
                # overwrites b=row4), then wp in place over row3
                # b is host-negated, so (-b)*sneg = b*sigma
                bsig_ = tp.tile([P, FC], f16, tag=f"bsig{g}")
                E[C["bsig"]].tensor_tensor(bsig_, pop_[:, 4, :], sneg_,
                                           op=Alu.mult)
                E[C["wm"]].tensor_tensor(pop_[:, 4, :], pop_[:, 3, :], bsig_,
                                         op=Alu.subtract)
                E[C["wp"]].tensor_tensor(pop_[:, 3, :], pop_[:, 3, :], bsig_,
                                         op=Alu.add)
                return dict(SM=SM_)

            pp, pop = fetch(0)
            # step-0 dots straight from the staging tile -- no wait on the
            # LS seed copy (which only the step-1 bulk needs)
            dots = tp.tile([P, 2, 2, FC], f16, tag="dots0")
            nc.vector.tensor_tensor(
                dots.rearrange("p a b f -> p (a b f)"),
                _ap(pp, 0, [[FC, 2], [0, 2], [1, FC]]),
                _ap(seed, 0, [[0, 2], [FC, 2], [1, FC]]), op=Alu.mult)
            A = waveA(0, dots, pop)
            # step-0 dif seeds the cross-iteration pairA tile
            pairA = tp.tile([P, 2, FC], f16, tag="pairA0")  # [lmul, dif]
            nc.vector.tensor_tensor(pairA[:, 1, :], dots[:, 0, 0, :],
                                    dots[:, 1, 0, :], op=Alu.subtract)
            lnew32 = None
            for s in range(D - 1):   # the last step is split separately
                w = s + 1
                t = "s"              # shared tags -> rotating buffers

                l1 = dots[:, 0, 0, :]
                s1 = dots[:, 0, 1, :]
                l2 = dots[:, 1, 0, :]
                s2 = dots[:, 1, 1, :]
                SM = A["SM"]
                dif = pairA[:, 1, :]

                pp_nxt, pop_nxt = fetch(s + 1)
                dots_nxt = tp.tile([P, 2, 2, FC], f16,
                                   tag=f"dots{(s + 1) % 2}")

                # ---- ring head (emission order = scheduler priority:
                # ring ops first, filler later)
                adif = tp.tile([P, FC], f16, tag=f"adif{t}")
                # |dif| via fp16 sign-bit mask (abs_max is not a valid
                # walrus TS alu op); int16 TS still gets the 4x DVE mode
                E[C["adif"]].tensor_scalar(adif.bitcast(i16),
                                           dif.bitcast(i16), 0x7FFF, None,
                                           op0=Alu.bitwise_and)
                pairB = tp.tile([P, 2, FC], f16, tag=f"pairB{t}")  # [e_u,-ec]
                nc.scalar.activation(pairB[:, 0, :], adif, Act.Exp,
                                     scale=-1.0)
                E[C["ecn"]].tensor_scalar(pairB[:, 1, :], pairB[:, 0, :],
                                          -1.0, -E_HI,
                                          op0=Alu.mult, op1=Alu.max)
                # pairC rows split: sp needs only e_u, so it runs on ACT
                # during the ecn DVE round-trip; lg follows ecn
                pairC = tp.tile([P, 2, FC], f16, tag=f"pairC{t}")  # [sp, lg]
                nc.scalar.activation(pairC[:, 0, :], pairB[:, 0, :],
                                     Act.Ln, bias=1.0, scale=1.0)
                nc.scalar.activation(pairC[:, 1, :], pairB[:, 1, :],
                                     Act.Ln, bias=1.0, scale=1.0)
                E[C["lmul"]].tensor_tensor(pairA[:, 0, :], l1, l2, op=Alu.add)
                TM = tp.tile([P, 2, FC], f16, tag=f"TM{t}")  # [TM2, TM3]
                nc.scalar.activation(TM.rearrange("p a f -> p (a f)"),
                                     pairA.rearrange("p a f -> p (a f)"),
                                     Act.Tanh, scale=INV_LIM)
                mx = tp.tile([P, FC], f16, tag=f"mx{t}")
                E[C["mx"]].tensor_tensor(mx, l1, l2, op=Alu.max)
                pairD = tp.tile([P, 2, FC], f16, tag=f"pairD{t}")
                E[C["prD"]].tensor_tensor(
                    pairD.rearrange("p a f -> p (a f)"),
                    _ap(mx, 0, [[0, 2], [1, FC]]),
                    pairC.rearrange("p a f -> p (a f)"), op=Alu.add)
                # pairE split into separate tiles: tb first (it unblocks
                # m4 without the double clip); t1 then its second tanh
                tb = tp.tile([P, FC], f16, tag=f"tb{t}")
                nc.scalar.activation(tb, pairD[:, 1, :],
                                     Act.Tanh, scale=INV_LIM)
                t1 = tp.tile([P, FC], f16, tag=f"t1{t}")
                nc.scalar.activation(t1, pairD[:, 0, :],
                                     Act.Tanh, scale=INV_LIM)
                # double clip of the same-sign branch: t1 -> tanh(t1)
                t1t = tp.tile([P, FC], f16, tag=f"t1t{t}")
                nc.scalar.activation(t1t, t1, Act.Tanh)
                # early l-mix rows: [c2*TM2, c3*TM3] + c4*l1
                mle = tp.tile([P, 2, FC], f16, tag=f"mle{t}")
                E[C["mle"]].tensor_tensor(
                    mle.rearrange("p a f -> p (a f)"),
                    _ap(pop, 0, [[FC, 2], [1, FC]]),
                    TM.rearrange("p a f -> p (a f)"), op=Alu.mult)
                ml2 = tp.tile([P, FC], f16, tag=f"ml2{t}")
                E[C["ml2"]].tensor_tensor(ml2, pop[:, 2, :], l1, op=Alu.mult)
                le1 = tp.tile([P, FC], f16, tag=f"le1{t}")
                E[C["le1"]].tensor_tensor(le1, mle[:, 0, :], ml2, op=Alu.add)
                le = tp.tile([P, FC], f16, tag=f"le{t}")
                E[C["le"]].tensor_tensor(le, le1, mle[:, 1, :], op=Alu.add)
                m4 = tp.tile([P, FC], f16, tag=f"m4{t}")
                E[C["m34"]].tensor_tensor(m4, pop[:, 4, :], tb, op=Alu.mult)
                la1 = tp.tile([P, FC], f16, tag=f"la1{t}")
                E[C["la1"]].tensor_tensor(la1, le, m4, op=Alu.add)
                m3 = tp.tile([P, FC], f16, tag=f"m3{t}")
                E[C["m34"]].tensor_tensor(m3, pop[:, 3, :], t1t, op=Alu.mult)
                lacc = tp.tile([P, FC], f16, tag=f"lacc{t}")
                E[C["lacc"]].tensor_tensor(lacc, la1, m3, op=Alu.add)

                # ---- RMS chain
                # ln(ms) folded: ln(srt/(s+2)); the reference's +1e-6 only
                # matters when ms is tiny, where scl = 1 either way
                tmix = tp.tile([P, FC], f16, tag=f"tmix{t}")
                nc.scalar.activation(tmix, lacc, Act.Tanh)
                Lx = tp.tile([P, FC], f16, tag=f"Lx{t}")
                E[C["Lx"]].tensor_scalar(Lx, tmix, LOG_LIM, None,
                                         op0=Alu.mult)
                Lsq = tp.tile([P, FC], f16, tag=f"Lsq{t}")
                E[C["Lsq"]].tensor_tensor(Lsq, Lx, Lx, op=Alu.mult)
                srt = tp.tile([P, FC], f16, tag=f"srt{t}")
                E[C["srt"]].tensor_tensor(srt, Lsq, ssq, op=Alu.add)
                lnms = tp.tile([P, FC], f16, tag=f"lnms{t}")
                nc.scalar.activation(lnms, srt, Act.Ln,
                                     scale=1.0 / (s + 2))
                dsq = tp.tile([P, FC], f16, tag=f"dsq{t}")
                nc.scalar.activation(dsq, lnms, Act.Exp, scale=-0.5)
                scl = tp.tile([P, FC], f16, tag=f"scl{t}")
                E[C["scl"]].tensor_scalar(scl, dsq, LOG_LIM, 1.0,
                                          op0=Alu.mult, op1=Alu.min)
                # lnew writes the LS state row directly (the sign side
                # already does this via st2) -- no mirror copy needed
                lnew = LS[:, 0, s + 1, :]
                E[C["lnew"]].tensor_tensor(lnew, Lx, scl, op=Alu.mult)

                # ---- next-step ring head: dif(s+1) = (part_l1 - part_l2)
                # + (p1-p2)[newest]*lnew, skipping the full corrL add.
                # lnew = Lx*scl is re-associated as (pdiff*Lx)*scl so the
                # ring goes scl -> crd -> dif without waiting for lnew.
                if True:
                    part_nxt = bulk(s + 1, pp_nxt)
                    pairA_nxt = tp.tile([P, 2, FC], f16,
                                        tag=f"pairA{(s + 1) % 2}")
                    dd = tp.tile([P, FC], f16, tag=f"dd{(s + 1) % 2}")
                    E[C["dd"]].tensor_tensor(
                        dd, _ap(part_nxt[0], part_nxt[1], [[1, FC]]),
                        _ap(part_nxt[0], part_nxt[1] + 2 * FC, [[1, FC]]),
                        op=Alu.subtract)
                    pdL = tp.tile([P, FC], f16, tag=f"pdL{(s + 1) % 2}")
                    E[C["crd"]].tensor_tensor(pdL, pop_nxt[:, 7, :], Lx,
                                              op=Alu.mult)
                    crd = tp.tile([P, FC], f16, tag=f"crd{(s + 1) % 2}")
                    E[C["crd"]].tensor_tensor(crd, pdL, scl, op=Alu.mult)
                    E[C["dif"]].tensor_tensor(pairA_nxt[:, 1, :], dd, crd,
                                              op=Alu.add)

                # ---- sign path + s-mix (off-ring, fills early stalls)
                cb = tp.tile([P, FC], f16, tag=f"cb{t}")
                E[C["cb"]].tensor_scalar(cb, dif, 0.0, None, op0=Alu.is_ge)
                nc.vector.copy_predicated(out=SM[:, 1, :],
                                          mask=cb.bitcast(i16), data=s1)
                ms = tp.tile([P, 4, FC], f16, tag=f"ms{t}")
                E[C["msa"]].tensor_tensor(
                    _ap(ms, 0, [[FC, 3], [1, FC]]),
                    _ap(pop, 3 * FC, [[FC, 3], [1, FC]]),
                    _ap(SM, 0, [[FC, 3], [1, FC]]), op=Alu.mult)
                E[C["msb"]].tensor_tensor(ms[:, 3, :], pop[:, 6, :], s1,
                                          op=Alu.mult)
                st = tp.tile([P, 2, FC], f16, tag=f"st{t}")
                E[C["st1"]].tensor_tensor(
                    st.rearrange("p a f -> p (a f)"),
                    _ap(ms, 0, [[FC, 2], [1, FC]]),
                    _ap(ms, 2 * FC, [[FC, 2], [1, FC]]), op=Alu.add)
                E[C["st2"]].tensor_tensor(LS[:, 1, s + 1, :], st[:, 0, :],
                                          st[:, 1, :], op=Alu.add)

                # ---- corr into next dots + state upkeep
                if True:
                    cin0 = _ap(pp_nxt, (s + 1) * FC,
                               [[(s + 2) * FC, 2], [1, FC]])
                    corrS = tp.tile([P, 2, FC], f16, tag=f"crS{(s + 1) % 2}")
                    E[C["crSp"]].tensor_tensor(
                        corrS.rearrange("p a f -> p (a f)"), cin0,
                        _ap(LS, N * FC + (s + 1) * FC, [[0, 2], [1, FC]]),
                        op=Alu.mult)
                    E[C["crSa"]].tensor_tensor(
                        _ap(dots_nxt, FC, [[2 * FC, 2], [1, FC]]),
                        _ap(part_nxt[0], part_nxt[1] + FC,
                            [[2 * FC, 2], [1, FC]]),
                        corrS.rearrange("p a f -> p (a f)"), op=Alu.add)
                    sqn = tp.tile([P, FC], f16, tag=f"sqn{t}")
                    E[C["ssqa"]].tensor_tensor(sqn, lnew, lnew, op=Alu.mult)
                    E[C["ssqa"]].tensor_tensor(ssq, ssq, sqn, op=Alu.add)
                    # log half of next step's dots straight from lnew
                    corrL = tp.tile([P, 2, FC], f16, tag=f"crL{(s + 1) % 2}")
                    E[C["crLp"]].tensor_tensor(
                        corrL.rearrange("p a f -> p (a f)"), cin0,
                        _ap(LS, (s + 1) * FC, [[0, 2], [1, FC]]),
                        op=Alu.mult)
                    E[C["crLa"]].tensor_tensor(
                        _ap(dots_nxt, 0, [[2 * FC, 2], [1, FC]]),
                        _ap(part_nxt[0], part_nxt[1],
                            [[2 * FC, 2], [1, FC]]),
                        corrL.rearrange("p a f -> p (a f)"), op=Alu.add)
                    A = waveA(s + 1, dots_nxt, pop_nxt)

                if probe_qs:
                    qmap = {
                        "l1": l1, "s1": s1, "l2": l2, "s2": s2,
                        "dif": dif, "mx": mx, "adif": adif,
                        "eu": pairB[:, 0, :],
                        "sp": pairC[:, 0, :], "lg": pairC[:, 1, :],
                        "t1": t1t, "tb": tb,
                        "tm2": TM[:, 0, :], "tm3": TM[:, 1, :],
                        "sm0": SM[:, 0, :], "sm1": SM[:, 1, :],
                        "s12": SM[:, 2, :],
                        "le": le, "lacc": lacc, "tmix": tmix, "scl": scl,
                        "lnew": lnew,
                        "smix": LS[:, 1, s + 1, :],
                        "ssq": ssq,
                    }
                    for qi, qn in enumerate(probe_qs):
                        pt = tp.tile([P, FC], f32, tag=f"pr{qn}{t}")
                        nc.vector.tensor_copy(out=pt, in_=qmap[qn])
                        nc.sync.dma_start(out=probe_d[qi * D + s], in_=pt)

                pp, pop, dots, pairA = pp_nxt, pop_nxt, dots_nxt, pairA_nxt

            # ---- final step (s = D-1), fp32 tail, split into two
            # half-token chains so the DVE/ACT ping-pong of one half
            # overlaps the other and the out DMA overlaps compute
            s = D - 1
            SM = A["SM"]
            NSP = int(os.environ.get("DAG2_FSP", "2"))
            HF = FC // NSP
            for hi, f0 in enumerate(range(0, FC, HF)):
                h = f"h{hi}"
                fsl = slice(f0, f0 + HF)
                l1 = _ap(dots, 0 * FC + f0, [[1, HF]])
                s1 = _ap(dots, 1 * FC + f0, [[1, HF]])
                l2 = _ap(dots, 2 * FC + f0, [[1, HF]])
                dif = pairA[:, 1, fsl]
                adif = tp.tile([P, HF], f16, tag=f"adif{h}")
                E[C["adif"]].tensor_scalar(adif.bitcast(i16),
                                           dif.bitcast(i16), 0x7FFF, None,
                                           op0=Alu.bitwise_and)
                pairB = tp.tile([P, 2, HF], f16, tag=f"pairB{h}")
                nc.scalar.activation(pairB[:, 0, :], adif, Act.Exp,
                                     scale=-1.0)
                E[C["ecn"]].tensor_scalar(pairB[:, 1, :], pairB[:, 0, :],
                                          -1.0, -E_HI,
                                          op0=Alu.mult, op1=Alu.max)
                pairC = tp.tile([P, 2, HF], f16, tag=f"pairC{h}")
                nc.scalar.activation(pairC[:, 0, :], pairB[:, 0, :],
                                     Act.Ln, bias=1.0, scale=1.0)
                nc.scalar.activation(pairC[:, 1, :], pairB[:, 1, :],
                                     Act.Ln, bias=1.0, scale=1.0)
                E[C["lmul"]].tensor_tensor(pairA[:, 0, fsl], l1, l2,
                                           op=Alu.add)
                TM = tp.tile([P, 2, HF], f16, tag=f"TM{h}")
                nc.scalar.activation(TM.rearrange("p a f -> p (a f)"),
                                     _ap(pairA, f0, [[FC, 2], [1, HF]]),
                                     Act.Tanh, scale=INV_LIM)
                mx = tp.tile([P, HF], f16, tag=f"mx{h}")
                E[C["mx"]].tensor_tensor(mx, l1, l2, op=Alu.max)
                pairD = tp.tile([P, 2, HF], f16, tag=f"pairD{h}")
                E[C["prD"]].tensor_tensor(
                    pairD.rearrange("p a f -> p (a f)"),
                    _ap(mx, 0, [[0, 2], [1, HF]]),
                    pairC.rearrange("p a f -> p (a f)"), op=Alu.add)
                tb = tp.tile([P, HF], f16, tag=f"tb{h}")
                nc.scalar.activation(tb, pairD[:, 1, :],
                                     Act.Tanh, scale=INV_LIM)
                t1 = tp.tile([P, HF], f16, tag=f"t1{h}")
                nc.scalar.activation(t1, pairD[:, 0, :],
                                     Act.Tanh, scale=INV_LIM)
                t1t = tp.tile([P, HF], f16, tag=f"t1t{h}")
                nc.scalar.activation(t1t, t1, Act.Tanh)
                mle = tp.tile([P, 2, HF], f16, tag=f"mle{h}")
                E[C["mle"]].tensor_tensor(
                    mle.rearrange("p a f -> p (a f)"),
                    _ap(pop, f0, [[FC, 2], [1, HF]]),
                    TM.rearrange("p a f -> p (a f)"), op=Alu.mult)
                ml2 = tp.tile([P, HF], f16, tag=f"ml2{h}")
                E[C["ml2"]].tensor_tensor(ml2, pop[:, 2, fsl], l1,
                                          op=Alu.mult)
                le1 = tp.tile([P, HF], f16, tag=f"le1{h}")
                E[C["le1"]].tensor_tensor(le1, mle[:, 0, :], ml2,
                                          op=Alu.add)
                le = tp.tile([P, HF], f16, tag=f"le{h}")
                E[C["le"]].tensor_tensor(le, le1, mle[:, 1, :], op=Alu.add)
                m4 = tp.tile([P, HF], f16, tag=f"m4{h}")
                E[C["m34"]].tensor_tensor(m4, pop[:, 4, fsl], tb,
                                          op=Alu.mult)
                la1 = tp.tile([P, HF], f16, tag=f"la1{h}")
                E[C["la1"]].tensor_tensor(la1, le, m4, op=Alu.add)
                m3 = tp.tile([P, HF], f16, tag=f"m3{h}")
                E[C["m34"]].tensor_tensor(m3, pop[:, 3, fsl], t1t,
                                          op=Alu.mult)
                lacc = tp.tile([P, HF], f16, tag=f"lacc{h}")
                E[C["lacc"]].tensor_tensor(lacc, la1, m3, op=Alu.add)
                # fp16 L-form tail (same as the loop steps; the fp16
                # quantum on lnew8 costs <1e-3 extra output rel err, and
                # e8 = Exp reads fp16 -> fp32 so exp itself stays fp32)
                tmix = tp.tile([P, HF], f16, tag=f"tmixF{h}")
                nc.scalar.activation(tmix, lacc, Act.Tanh)
                Lxf = tp.tile([P, HF], f16, tag=f"LxF{h}")
                E[C["Lx"]].tensor_scalar(Lxf, tmix, LOG_LIM, None,
                                         op0=Alu.mult)
                Lsqf = tp.tile([P, HF], f16, tag=f"LsqF{h}")
                E[C["Lsq"]].tensor_tensor(Lsqf, Lxf, Lxf, op=Alu.mult)
                srt = tp.tile([P, HF], f16, tag=f"srtF{h}")
                E[C["srtF"]].tensor_tensor(srt, Lsqf, ssq[:, fsl],
                                           op=Alu.add)
                lnms = tp.tile([P, HF], f16, tag=f"lnmsF{h}")
                nc.scalar.activation(lnms, srt, Act.Ln,
                                     scale=1.0 / (s + 2))
                dsq = tp.tile([P, HF], f16, tag=f"dsqF{h}")
                nc.scalar.activation(dsq, lnms, Act.Exp, scale=-0.5)
                scl = tp.tile([P, HF], f16, tag=f"sclF{h}")
                E[C["sclF"]].tensor_scalar(scl, dsq, LOG_LIM, 1.0,
                                           op0=Alu.mult, op1=Alu.min)
                lnew32 = tp.tile([P, HF], f16, tag=f"lnewF{h}")
                E[C["lnewF"]].tensor_tensor(lnew32, Lxf, scl, op=Alu.mult)
                # sign path + s-mix for this half
                cb = tp.tile([P, HF], f16, tag=f"cb{h}")
                E[C["cb"]].tensor_scalar(cb, dif, 0.0, None, op0=Alu.is_ge)
                nc.vector.copy_predicated(out=SM[:, 1, fsl],
                                          mask=cb.bitcast(i16), data=s1)
                ms = tp.tile([P, 4, HF], f16, tag=f"ms{h}")
                E[C["msa"]].tensor_tensor(
                    _ap(ms, 0, [[HF, 3], [1, HF]]),
                    _ap(pop, 3 * FC + f0, [[FC, 3], [1, HF]]),
                    _ap(SM, f0, [[FC, 3], [1, HF]]), op=Alu.mult)
                E[C["msb"]].tensor_tensor(ms[:, 3, :], pop[:, 6, fsl], s1,
                                          op=Alu.mult)
                st = tp.tile([P, 2, HF], f16, tag=f"st{h}")
                E[C["st1"]].tensor_tensor(
                    st.rearrange("p a f -> p (a f)"),
                    _ap(ms, 0, [[HF, 2], [1, HF]]),
                    _ap(ms, 2 * HF, [[HF, 2], [1, HF]]), op=Alu.add)
                smix = tp.tile([P, HF], f16, tag=f"smix{h}")
                E[C["st2"]].tensor_tensor(smix, st[:, 0, :], st[:, 1, :],
                                          op=Alu.add)
                # out = smix * exp(lnew)
                e8 = tp.tile([P, HF], f32, tag=f"e8{h}")
                nc.scalar.activation(e8, lnew32, Act.Exp)
                ot = tp.tile([P, HF], f32, tag=f"ot{h}")
                E[C["ot"]].tensor_tensor(ot, smix, e8, op=Alu.mult)
                nc.sync.dma_start(out=out_d[0, :, f0:f0 + HF], in_=ot)

    _split_waits(nc, 1)
    return nc


_BUILD_CACHE = {}


def _get_nc():
    if "nc" not in _BUILD_CACHE:
        _BUILD_CACHE["nc"] = _build()
    return _BUILD_CACHE["nc"]


def _pack_inputs(initial_sgn, initial_log, p1, p2, pop):
    """Host-side sharding + fp16 packing.  Token layout:
    flat token = c*TOK_CORE + p*FC + f."""
    in_maps = []
    sg = initial_sgn.reshape(NCORE, P, FC, N)
    lg = initial_log.reshape(NCORE, P, FC, N)
    p1s = p1.reshape(NCORE, P, FC, D, N)
    p2s = p2.reshape(NCORE, P, FC, D, N)
    pops = pop.reshape(NCORE, P, FC, D, 5)
    for c in range(NCORE):
        # pp: per step [P, 2, w, FC] fp16 (i-major, slot-major, f inner)
        blocks = []
        for s in range(D):
            w = s + 1
            blk = np.stack([p1s[c, :, :, s, :w], p2s[c, :, :, s, :w]],
                           axis=1)                       # [P, 2, FC, w]
            blocks.append(blk.transpose(0, 1, 3, 2)      # [P, 2, w, FC]
                          .reshape(P, 2 * w * FC))
        pp_arr = np.ascontiguousarray(
            np.concatenate(blocks, axis=1)).astype(np.float16)[None]

        q = pops[c].transpose(2, 0, 3, 1)                # [D, P, 5, FC]
        a = (q[:, :, 0] + q[:, :, 1]) * 0.5
        b = (q[:, :, 1] - q[:, :, 0]) * 0.5   # pre-negated: -(p0-p1)/2
        c2 = q[:, :, 2]
        c3 = q[:, :, 3]
        c4 = q[:, :, 4] * np.float32(INV_LIM)
        c23 = q[:, :, 2] + q[:, :, 3]
        c4s = q[:, :, 4]
        # (p1-p2) on the newest slot of each step, for the dif shortcut
        pd = np.stack([p1s[c, :, :, s, s] - p2s[c, :, :, s, s]
                       for s in range(D)], axis=0)       # [D, P, FC]
        pop_arr = np.ascontiguousarray(
            np.stack([c2, c3, c4, a, b, c23, c4s, pd], axis=2)
            .reshape(D, P, 8 * FC)).astype(np.float16)

        ls16 = np.ascontiguousarray(
            np.stack([lg[c, :, :, 0], sg[c, :, :, 0]], axis=1)
            .reshape(P, 2 * FC)).astype(np.float16)
        in_maps.append({"pp": pp_arr, "pop": pop_arr,
                        "ls16": ls16[None]})
    return in_maps


def kernel(initial_sgn, initial_log, operand1_probs, operand2_probs,
           operation_probs):
    initial_sgn = np.ascontiguousarray(initial_sgn, dtype=np.float32)
    initial_log = np.ascontiguousarray(initial_log, dtype=np.float32)
    p1 = np.asarray(operand1_probs, dtype=np.float32)
    p2 = np.asarray(operand2_probs, dtype=np.float32)
    pop = np.asarray(operation_probs, dtype=np.float32)

    assert not initial_sgn[..., 1:].any() and not initial_log[..., 1:].any(), \
        "fast path requires zero-initialized scratch slots 1..8"

    nc = _get_nc()
    in_maps = _pack_inputs(initial_sgn, initial_log, p1, p2, pop)
    res = run_bass_kernel_spmd(nc, in_maps, core_ids=list(range(NCORE)))
    out = np.stack([r["out"] for r in res.results], axis=0)
    out = out.reshape(B, T)
    return np.ascontiguousarray(out)
